# revision 2
# baseline (speedup 1.0000x reference)
"""Trainium2 Bass kernel for nn_Attn: softmax(enc @ (W^T h)) over seq_len.

Math: energy = enc @ W^T + b; attn = energy @ h; out = softmax(attn).
Algebraically attn[s] = enc[s,:] . v + (b.h) with v = W^T h, and the (b.h)
term is constant across s so softmax cancels it. The device work is the
memory-bound part: streaming the 128 MiB encoder_outputs once, sharded
along seq_len across 8 NeuronCores, computing per-row dots against v via
fused DVE tensor_tensor_reduce.
"""
import numpy as np

S = 32768
H = 1024
N_CORES = 8
S_SHARD = S // N_CORES          # 4096 rows per core
P = 128                         # partitions
N_BLK = S_SHARD // P            # 32 row-blocks per core

_cache = {}


def _build():
    from concourse import bacc, mybir, tile

    nc = bacc.Bacc("TRN2", target_bir_lowering=False, debug=False,
                   num_devices=N_CORES)
    enc = nc.dram_tensor("enc", [S_SHARD, H], mybir.dt.float32,
                         kind="ExternalInput")
    vrep = nc.dram_tensor("vrep", [P, H], mybir.dt.float32,
                          kind="ExternalInput")
    e_out = nc.dram_tensor("e_out", [P, N_BLK], mybir.dt.float32,
                           kind="ExternalOutput")

    with tile.TileContext(nc) as tc:
        with tc.tile_pool(name="const", bufs=1) as cpool, \
             tc.tile_pool(name="stream", bufs=4) as spool, \
             tc.tile_pool(name="prod", bufs=3) as ppool, \
             tc.tile_pool(name="cpout", bufs=2) as opool:
            vt = cpool.tile([P, H], mybir.dt.float32)
            nc.sync.dma_start(out=vt[:], in_=vrep.ap())
            E = cpool.tile([P, N_BLK], mybir.dt.float32)
            for b in range(N_BLK):
                t = spool.tile([P, H], mybir.dt.float32, tag="t")
                nc.sync.dma_start(out=t[:], in_=enc.ap()[b * P:(b + 1) * P, :])
                prod = ppool.tile([P, H], mybir.dt.float32, tag="prod")
                # multiply on VectorE; reduce on ScalarE (accum_out) so the
                # two passes over the data run on different engines
                nc.vector.tensor_tensor(out=prod[:], in0=t[:], in1=vt[:],
                                        op=mybir.AluOpType.mult)
                cp = opool.tile([P, H], mybir.dt.float32, tag="cp")
                nc.scalar.activation(out=cp[:], in_=prod[:],
                                     func=mybir.ActivationFunctionType.Copy,
                                     accum_out=E[:, b:b + 1])
            nc.sync.dma_start(out=e_out.ap(), in_=E[:])
    nc.compile()
    return nc


def _get_nc():
    if "nc" not in _cache:
        _cache["nc"] = _build()
    return _cache["nc"]


def kernel(hidden, encoder_outputs, W, b):
    from concourse import bass_utils

    nc = _get_nc()
    h = np.asarray(hidden, dtype=np.float32)[0]
    enc = np.ascontiguousarray(np.asarray(encoder_outputs,
                                          dtype=np.float32)[:, 0, :])
    v = np.asarray(W, dtype=np.float32).T @ h          # v[h] = sum_o W[o,h] h[o]
    vrep = np.ascontiguousarray(np.broadcast_to(v[None, :], (P, H)))

    in_maps = [{"enc": enc[c * S_SHARD:(c + 1) * S_SHARD], "vrep": vrep}
               for c in range(N_CORES)]
    res = bass_utils.run_bass_kernel_spmd(
        nc, in_maps, core_ids=list(range(N_CORES)),
        trace=_cache.get("trace", False))
    _cache["last_result"] = res

    # e_out is [partition, block]; global row s = core*4096 + block*128 + p.
    e = np.concatenate(
        [res.results[c]["e_out"].T.reshape(S_SHARD) for c in range(N_CORES)])
    e = e - e.max()
    p = np.exp(e)
    out = (p / p.sum()).astype(np.float32)
    return out[None, None, :]


# revision 3
# speedup vs baseline: 1.0506x; 1.0506x over previous
"""Trainium2 Bass kernel for nn_Attn: softmax(enc @ (W^T h)) over seq_len.

Math: energy = enc @ W^T + b; attn = energy @ h; out = softmax(attn).
Algebraically attn[s] = enc[s,:] . v + (b.h) with v = W^T h, and the (b.h)
term is constant across s so softmax cancels it. The device work is the
memory-bound part: streaming the 128 MiB encoder_outputs once, sharded
along seq_len across 8 NeuronCores. Per 128-row block: VectorE multiplies
by v (tensor_tensor), ScalarE reduces rows (activation Copy + accum_out),
so the two passes over the data run on different engines concurrently.
"""
import numpy as np

S = 32768
H = 1024
N_CORES = 8
S_SHARD = S // N_CORES          # 4096 rows per core
P = 128                         # partitions
N_BLK = S_SHARD // P            # 32 row-blocks per core
BLK_PER_DMA = 2                 # 1 MiB per dma_start
N_DVE_REDUCE = 3                # reduces stolen from ScalarE to balance engines

_cache = {}


def _build():
    from concourse import bacc, mybir, tile

    nc = bacc.Bacc("TRN2", target_bir_lowering=False, debug=False,
                   num_devices=N_CORES)
    enc = nc.dram_tensor("enc", [S_SHARD, H], mybir.dt.float32,
                         kind="ExternalInput")
    v_in = nc.dram_tensor("v_in", [1, H], mybir.dt.float32,
                          kind="ExternalInput")
    e_out = nc.dram_tensor("e_out", [P, N_BLK], mybir.dt.float32,
                           kind="ExternalOutput")

    with tile.TileContext(nc) as tc:
        with tc.tile_pool(name="const", bufs=1) as cpool, \
             tc.tile_pool(name="stream", bufs=6) as spool, \
             tc.tile_pool(name="prod", bufs=4) as ppool, \
             tc.tile_pool(name="cpout", bufs=4) as opool:
            v0 = cpool.tile([1, H], mybir.dt.float32)
            nc.sync.dma_start(out=v0[:], in_=v_in.ap())
            vt = cpool.tile([P, H], mybir.dt.float32)
            nc.gpsimd.partition_broadcast(vt[:], v0[0:1, :])
            E = cpool.tile([P, N_BLK], mybir.dt.float32)
            for d in range(N_BLK // BLK_PER_DMA):
                t = spool.tile([P, BLK_PER_DMA, H], mybir.dt.float32, tag="t")
                rows = enc.ap()[d * BLK_PER_DMA * P:(d + 1) * BLK_PER_DMA * P, :]
                nc.sync.dma_start(out=t[:], in_=rows.rearrange(
                    "(i p) h -> p i h", p=P))
                for i in range(BLK_PER_DMA):
                    b = d * BLK_PER_DMA + i
                    prod = ppool.tile([P, H], mybir.dt.float32, tag="prod")
                    nc.vector.tensor_tensor(out=prod[:], in0=t[:, i, :],
                                            in1=vt[:],
                                            op=mybir.AluOpType.mult)
                    if b % (N_BLK // N_DVE_REDUCE + 1) == N_BLK // N_DVE_REDUCE:
                        nc.vector.tensor_reduce(out=E[:, b:b + 1], in_=prod[:],
                                                axis=mybir.AxisListType.X,
                                                op=mybir.AluOpType.add)
                    else:
                        cp = opool.tile([P, H], mybir.dt.float32, tag="cp")
                        nc.scalar.activation(
                            out=cp[:], in_=prod[:],
                            func=mybir.ActivationFunctionType.Copy,
                            accum_out=E[:, b:b + 1])
            nc.sync.dma_start(out=e_out.ap(), in_=E[:])
    nc.compile()
    return nc


def _get_nc():
    if "nc" not in _cache:
        _cache["nc"] = _build()
    return _cache["nc"]


def kernel(hidden, encoder_outputs, W, b):
    from concourse import bass_utils

    nc = _get_nc()
    h = np.asarray(hidden, dtype=np.float32)[0]
    enc = np.ascontiguousarray(np.asarray(encoder_outputs,
                                          dtype=np.float32)[:, 0, :])
    v = (np.asarray(W, dtype=np.float32).T @ h).astype(np.float32)

    in_maps = [{"enc": enc[c * S_SHARD:(c + 1) * S_SHARD],
                "v_in": v[None, :]} for c in range(N_CORES)]
    res = bass_utils.run_bass_kernel_spmd(
        nc, in_maps, core_ids=list(range(N_CORES)),
        trace=_cache.get("trace", False))
    _cache["last_result"] = res

    # e_out is [partition, block]; global row s = core*4096 + block*128 + p.
    e = np.concatenate(
        [res.results[c]["e_out"].T.reshape(S_SHARD) for c in range(N_CORES)])
    e = e - e.max()
    p = np.exp(e)
    out = (p / p.sum()).astype(np.float32)
    return out[None, None, :]


# revision 6
# speedup vs baseline: 1.1929x; 1.1354x over previous
"""Trainium2 Bass kernel for nn_Attn: softmax(enc @ (W^T h)) over seq_len.

Math: energy = enc @ W^T + b; attn = energy @ h; out = softmax(attn).
Algebraically attn[s] = enc[s,:] . v + (b.h) with v = W^T h, and the (b.h)
term is constant across s so softmax cancels it. The device work is the
memory-bound part: streaming the 128 MiB encoder_outputs once, sharded
along seq_len across 8 NeuronCores. Per 128-row block: VectorE multiplies
by v (tensor_tensor), ScalarE reduces rows (activation Copy + accum_out),
so the two passes over the data run on different engines concurrently.
"""
import numpy as np

S = 32768
H = 1024
N_CORES = 8
S_SHARD = S // N_CORES          # 4096 rows per core
P = 128                         # partitions
N_BLK = S_SHARD // P            # 32 row-blocks per core
BLK_PER_DMA = 4                 # 2 MiB per dma_start
N_DVE_REDUCE = 3                # reduces stolen from ScalarE to balance engines

_cache = {}


def _build():
    from concourse import bacc, mybir, tile

    nc = bacc.Bacc("TRN2", target_bir_lowering=False, debug=False,
                   num_devices=N_CORES)
    enc = nc.dram_tensor("enc", [S_SHARD, H], mybir.dt.float32,
                         kind="ExternalInput")
    vrep = nc.dram_tensor("vrep", [P, H], mybir.dt.float32,
                          kind="ExternalInput")
    e_out = nc.dram_tensor("e_out", [P, N_BLK], mybir.dt.float32,
                           kind="ExternalOutput")

    with tile.TileContext(nc) as tc:
        with tc.tile_pool(name="const", bufs=1) as cpool, \
             tc.tile_pool(name="stream", bufs=4) as spool, \
             tc.tile_pool(name="prod", bufs=4) as ppool, \
             tc.tile_pool(name="cpout", bufs=4) as opool:
            vt = cpool.tile([P, H], mybir.dt.float32)
            nc.sync.dma_start(out=vt[:], in_=vrep.ap())
            E = cpool.tile([P, N_BLK], mybir.dt.float32)
            for d in range(N_BLK // BLK_PER_DMA):
                t = spool.tile([P, BLK_PER_DMA, H], mybir.dt.float32, tag="t")
                rows = enc.ap()[d * BLK_PER_DMA * P:(d + 1) * BLK_PER_DMA * P, :]
                nc.sync.dma_start(out=t[:], in_=rows.rearrange(
                    "(i p) h -> p i h", p=P))
                for i in range(BLK_PER_DMA):
                    b = d * BLK_PER_DMA + i
                    prod = ppool.tile([P, H], mybir.dt.float32, tag="prod")
                    nc.vector.tensor_tensor(out=prod[:], in0=t[:, i, :],
                                            in1=vt[:],
                                            op=mybir.AluOpType.mult)
                    if b % (N_BLK // N_DVE_REDUCE + 1) == N_BLK // N_DVE_REDUCE:
                        nc.vector.tensor_reduce(out=E[:, b:b + 1], in_=prod[:],
                                                axis=mybir.AxisListType.X,
                                                op=mybir.AluOpType.add)
                    else:
                        cp = opool.tile([P, H], mybir.dt.float32, tag="cp")
                        nc.scalar.activation(
                            out=cp[:], in_=prod[:],
                            func=mybir.ActivationFunctionType.Copy,
                            accum_out=E[:, b:b + 1])
            nc.sync.dma_start(out=e_out.ap(), in_=E[:])
    nc.compile()
    return nc


def _get_nc():
    if "nc" not in _cache:
        _cache["nc"] = _build()
    return _cache["nc"]


def kernel(hidden, encoder_outputs, W, b):
    from concourse import bass_utils

    nc = _get_nc()
    h = np.asarray(hidden, dtype=np.float32)[0]
    enc = np.ascontiguousarray(np.asarray(encoder_outputs,
                                          dtype=np.float32)[:, 0, :])
    v = (np.asarray(W, dtype=np.float32).T @ h).astype(np.float32)
    vrep = np.ascontiguousarray(np.broadcast_to(v[None, :], (P, H)))

    in_maps = [{"enc": enc[c * S_SHARD:(c + 1) * S_SHARD],
                "vrep": vrep} for c in range(N_CORES)]
    res = bass_utils.run_bass_kernel_spmd(
        nc, in_maps, core_ids=list(range(N_CORES)),
        trace=_cache.get("trace", False))
    _cache["last_result"] = res

    # e_out is [partition, block]; global row s = core*4096 + block*128 + p.
    e = np.concatenate(
        [res.results[c]["e_out"].T.reshape(S_SHARD) for c in range(N_CORES)])
    e = e - e.max()
    p = np.exp(e)
    out = (p / p.sum()).astype(np.float32)
    return out[None, None, :]
